# revision 51
# baseline (speedup 1.0000x reference)
"""CapsuleLayer (dynamic routing) Trainium2 kernel — 8 NeuronCores, SPMD.

Strategy: shard the input-capsule axis IC=9216 across 8 cores (1152 each).
Per core, the weight shard (2.95 MB bf16) and both x layouts (2.4 MB bf16)
stay resident in SBUF, so u_hat ([64,9216,10,16] = 377 MB fp32) is never
materialized in HBM — it is recomputed on the tensor engine as needed.

Per routing iteration (3 total, unrolled):
  s~_j   = sum_i exp(b_ij) * u_hat[b,i,j,s]   -> per-core partial via 72
           PSUM-accumulated matmuls over K=(128 i's) x (8 u's)
  Z_j    = sum_i exp(b_ij)                     -> softmax normalizer partial
  ONE AllReduce carries [s~ partial (64x160) ; Z partial] (41.6 KB fp32) in
  a single packed [65,SJ] buffer (one DMA each way; the tiny Z row is DMA'd
  first on the way back so the Z-broadcast matmul overlaps the s~ DMA).
  v = squash(s~/Z) is computed identically on every core; the agreement
  update u_vj = mean_b <u_hat, v> is purely local to the core's i-shard:
  T'[i,u,s,j] = sum_b x[b,u,i] * (v[b,j,s]/B)  (72 K=64 matmuls)
  u_vj[i,j]   = sum_{u,s} W[i,j,s,u] * T'      (DVE multiply + one add +
              a single two-axis (u,s) tensor_reduce per i-tile)
Iteration 1 uses the exact uniform softmax c=1/IC (folded into the PSUM
pack copy), iteration 3 skips the dead agreement update.

Timing model (measured): the collectives-firmware boot makes the first
AllReduce complete at ~75-90us regardless of trigger time (boot variance
dominates run-to-run spread).  Each later iteration is DVE-bound
(~26us: exp(b)*W scaling, W*T' multiply + u/s reduction tree) with the
72+72 matmuls and PSUM copies hidden under it; each CC AllReduce costs
~14us end-to-end (fixed ~10.5us firmware op + handoffs).  A short PE
warm-up burst keeps the tensor engine out of the cold p-state for the
iter-0 weighted sum.  Remote-DMA mesh exchange (bypassing ncfw) was
prototyped and is correct but loses: per-NEFF launch skew (ms-scale,
absorbed only by CC-containing NEFFs) plus SWDGE dummy-descriptor
serialization (~8us/lane/transfer) make it slower than the CC op.
Measured: ~171-195us HW exec (boot variance), rel err 4.1e-3.
"""

import numpy as np
import ml_dtypes

B, IU, IC, NU, US = 64, 8, 9216, 10, 16
N_CORES = 8
S = IC // N_CORES
M9 = S // 128
SJ = US * NU
BF16 = ml_dtypes.bfloat16

_CACHE = {}


def _split_multi_waits(nc):
    import copy

    import bass_rust

    template = None
    for f in nc.m.functions:
        for blk in f.blocks:
            for inst in blk.instructions:
                if type(inst).__name__ == "InstEventSemaphore":
                    template = inst
                    break
            if template is not None:
                break
    assert template is not None, "no EventSemaphore template found"

    n = 0
    for f in nc.m.functions:
        for blk in f.blocks:
            out = []
            changed = False
            for inst in blk.instructions:
                si = inst.sync_info
                if si is not None and si.on_wait and len(si.on_wait) > 1:
                    waits = list(si.on_wait)
                    for w in waits[:-1]:
                        c = copy.deepcopy(template)
                        c.name = f"split_wait_{n}"
                        n += 1
                        c.engine = inst.engine
                        c.sync_info = bass_rust.SyncInfo(on_wait=[w], on_update=[])
                        out.append(c)
                    si.on_wait = [waits[-1]]
                    changed = True
                out.append(inst)
            if changed:
                blk.instructions = out


def _build_program():
    from concourse import bass, tile, mybir

    f32 = mybir.dt.float32
    bf16 = mybir.dt.bfloat16
    MUL = mybir.AluOpType.mult
    ADD = mybir.AluOpType.add

    nc = bass.Bass(
        "TRN2", target_bir_lowering=False, debug=False, num_devices=N_CORES
    )
    wa_in = nc.dram_tensor("wa", [128, M9, IU * US, NU], bf16, kind="ExternalInput").ap()
    xc_in = nc.dram_tensor("xc", [128, M9, IU, B], bf16, kind="ExternalInput").ap()
    xt_in = nc.dram_tensor("xt", [B, IU, S], bf16, kind="ExternalInput").ap()
    wb_in = nc.dram_tensor("wb", [128, M9, IU, SJ], bf16, kind="ExternalInput").ap()
    y_out = nc.dram_tensor("y", [B, NU, US], f32, kind="ExternalOutput").ap()

    with tile.TileContext(nc) as tc:
        with (
            tc.tile_pool(name="const", bufs=1) as cp,
            tc.tile_pool(name="work", bufs=8) as wp,
            tc.tile_pool(name="psum_s", bufs=1, space="PSUM") as pps,
            tc.tile_pool(name="psum_t", bufs=3, space="PSUM") as ppt,
            tc.tile_pool(name="psum_z", bufs=1, space="PSUM") as ppz,
            tc.tile_pool(name="dram", bufs=1, space="DRAM") as dp,
        ):
            # ---- resident tensors ----
            wa = cp.tile([128, M9, IU * US, NU], bf16, tag="wa")
            cw = cp.tile([128, M9, IU * US, NU], bf16, tag="cw")
            xc = cp.tile([128, M9, IU, B], bf16, tag="xc")
            xt = cp.tile([B, IU, S], bf16, tag="xt")
            wb2 = cp.tile([128, M9, IU, SJ], bf16, tag="wb2")
            ones = cp.tile([128, 1], f32, tag="ones")
            ones1 = cp.tile([1, B], f32, tag="ones1")
            b64 = cp.tile([B, 1], f32, tag="b64")
            tl1 = cp.tile([1, 2], f32, tag="tl1")
            tl2 = cp.tile([1, 2], f32, tag="tl2")
            b1c = cp.tile([B, 1], f32, tag="b1c")
            b_acc = cp.tile([128, M9, NU], f32, tag="bacc")
            e128 = cp.tile([128, M9, NU], bf16, tag="e128")
            uv = cp.tile([128, M9, NU], f32, tag="uv")
            zred = cp.tile([128, NU], f32, tag="zred")

            nc.sync.dma_start(out=wa[:], in_=wa_in[:])
            nc.sync.dma_start(out=xc[:], in_=xc_in[:])
            nc.sync.dma_start(out=xt[:], in_=xt_in[:])
            nc.sync.dma_start(out=wb2[:], in_=wb_in[:])
            nc.vector.memset(ones[:], 1.0)
            nc.vector.memset(ones1[:], 1.0)
            nc.vector.memset(b64[:], float(B))
            nc.vector.memset(tl1[:], 1.0)
            nc.scalar.sqrt(tl2[:], tl1[:])
            nc.vector.memset(b1c[:], 1.0)
            warm = cp.tile([128, 128], bf16, tag="warm")
            nc.vector.memset(warm[:], 0)
            pw = ppz.tile([2, 128], f32, tag="pzz")
            for _ in range(36):
                nc.tensor.matmul(pw[:], warm[:, 0:2], warm[:], start=True, stop=True)

            ar_bufs = []
            for it in range(3):
                ar_in = dp.tile([65, SJ], f32, tag=f"arin{it}")
                ar_out = dp.tile([65, SJ], f32, tag=f"arout{it}")
                ar_bufs.append((ar_in, ar_out))

            for it in range(3):
                ar_in, ar_out = ar_bufs[it]
                # ---- weighted-sum matmuls: s~ partial [64, (s,j)] ----
                rhs_src = wa if it == 0 else cw
                ps = pps.tile([B, US, NU], f32, tag="ps")
                n_mm = M9 * IU
                k = 0
                for m in range(M9):
                    if it > 0:
                        e_b = (
                            e128[:, m]
                            .unsqueeze(1)
                            .broadcast_to([128, IU * US, NU])
                        )
                        nc.vector.tensor_tensor(cw[:, m], wa[:, m], e_b, MUL)
                    for u in range(IU):
                        nc.tensor.matmul(
                            ps[:],
                            xc[:, m, u],
                            rhs_src[:, m, US * u : US * (u + 1)],
                            start=(k == 0),
                            stop=(k == n_mm - 1),
                        )
                        k += 1
                # pack [s~ ; Z] into one [65, SJ] tile, one DMA
                ars = wp.tile([65, US, NU], f32, tag="ars")
                if it == 0:
                    nc.scalar.mul(ars[0:B], ps[:], 1.0 / IC)
                else:
                    nc.scalar.copy(ars[0:B], ps[:])
                if it > 0:
                    nc.vector.tensor_reduce(
                        zred[:], e128[:].transpose([0, 2, 1]), mybir.AxisListType.X, ADD
                    )
                    pz = ppz.tile([1, NU], f32, tag="pzz")
                    nc.tensor.matmul(pz[:], ones[:], zred[:], start=True, stop=True)
                    nc.vector.tensor_copy(ars[B : B + 1, 0, 0:NU], pz[:])
                else:
                    nc.vector.memset(ars[B : B + 1], 0.0)
                nc.sync.dma_start(out=ar_in[:], in_=ars[:].rearrange("p s j -> p (s j)"))

                nc.gpsimd.collective_compute(
                    "AllReduce",
                    ADD,
                    replica_groups=[list(range(N_CORES))],
                    ins=[ar_in.opt()],
                    outs=[ar_out.opt()],
                )

                # ---- DMA out: tiny Z row first, then s~ block ----
                if it > 0:
                    zb1 = wp.tile([1, NU], f32, tag="zb1")
                    nc.sync.dma_start(out=zb1[:], in_=ar_out[B : B + 1, 0:NU])
                s_sb = wp.tile([B, US, NU], f32, tag="s")
                nc.sync.dma_start(
                    out=s_sb[:].rearrange("p s j -> p (s j)"), in_=ar_out[0:B]
                )
                if it == 0:
                    pass
                else:
                    pzb = pps.tile([B, NU], f32, tag="ps")
                    nc.tensor.matmul(
                        pzb[:], ones1[:], zb1[:],
                        start=True, stop=True,
                    )
                    rz = wp.tile([B, NU], f32, tag="rz")
                    nc.vector.reciprocal(rz[:], pzb[:])
                    nc.vector.tensor_tensor(
                        s_sb[:], s_sb[:],
                        rz[:].unsqueeze(1).broadcast_to([B, US, NU]), MUL,
                    )
                sq = wp.tile([B, US, NU], f32, tag="sq")
                nc.vector.tensor_tensor(sq[:], s_sb[:], s_sb[:], MUL)
                msq = wp.tile([B, US], f32, tag="msq")
                nc.vector.tensor_reduce(msq[:], sq[:], mybir.AxisListType.X, ADD)
                mroot = wp.tile([B, US], f32, tag="mroot")
                nc.scalar.sqrt(mroot[:], msq[:])
                den = wp.tile([B, US], f32, tag="den")
                nc.scalar.activation(
                    den[:], msq[:], mybir.ActivationFunctionType.Identity,
                    bias=(b64 if it < 2 else b1c)[:],
                    scale=float(B) if it < 2 else 1.0,
                )
                if it < 2:
                    nc.scalar.activation(
                        tl2[:], tl1[:], mybir.ActivationFunctionType.Exp
                    )
                rden = wp.tile([B, US], f32, tag="rden")
                nc.vector.reciprocal(rden[:], den[:])
                f_sb = wp.tile([B, US], f32, tag="f")
                nc.vector.tensor_tensor(f_sb[:], mroot[:], rden[:], MUL)

                if it < 2:
                    # ---- agreement update: local u_vj, b += ----
                    vB = wp.tile([B, NU, US], bf16, tag="vB")
                    nc.vector.tensor_tensor(
                        vB[:].transpose([0, 2, 1]),
                        s_sb[:],
                        f_sb[:].unsqueeze(2).broadcast_to([B, US, NU]),
                        MUL,
                    )
                    for m in range(M9):
                        tb = wp.tile([128, IU, SJ], bf16, tag="tb")
                        for h in range(2):
                            pt = ppt.tile([128, 4, 256], f32, tag="pt")
                            for kk in range(4):
                                u = 4 * h + kk
                                nc.tensor.matmul(
                                    pt[:, kk, 0:SJ],
                                    xt[:, u, 128 * m : 128 * (m + 1)],
                                    vB[:],
                                    start=True,
                                    stop=True,
                                )
                            nc.scalar.copy(
                                tb[:, 4 * h : 4 * (h + 1), :], pt[:, :, 0:SJ]
                            )
                        p_sb = wp.tile([128, IU, SJ], bf16, tag="p")
                        nc.vector.tensor_tensor(p_sb[:], wb2[:, m], tb[:], MUL)
                        uvt = b_acc if it == 0 else uv
                        t1 = wp.tile([128, 4, SJ], bf16, tag="t1")
                        nc.vector.tensor_tensor(
                            t1[:], p_sb[:, 0:4], p_sb[:, 4:8], ADD
                        )
                        nc.vector.tensor_reduce(
                            uvt[:, m],
                            t1[:].rearrange("p u (j s) -> p j u s", s=US),
                            mybir.AxisListType.XY,
                            ADD,
                        )
                        if it > 0:
                            nc.vector.tensor_tensor(
                                b_acc[:, m], b_acc[:, m], uv[:, m], ADD
                            )
                        nc.scalar.activation(
                            e128[:, m],
                            b_acc[:, m],
                            mybir.ActivationFunctionType.Exp,
                        )
                    nc.scalar.sqrt(tl2[:], tl1[:])
                else:
                    # ---- final output v = s * f, stored j-major ----
                    v2 = wp.tile([B, NU, US], f32, tag="v2")
                    nc.vector.tensor_tensor(
                        v2[:].transpose([0, 2, 1]),
                        s_sb[:],
                        f_sb[:].unsqueeze(2).broadcast_to([B, US, NU]),
                        MUL,
                    )
                    nc.sync.dma_start(out=y_out[:], in_=v2[:])
    _split_multi_waits(nc)
    return nc


def _shard_inputs(x, weight):
    w = np.asarray(weight).reshape(IC, NU, US, IU)
    x = np.asarray(x)
    wb = w.astype(BF16)
    xb = x.astype(BF16)
    in_maps = []
    for c in range(N_CORES):
        i0 = c * S
        ws = wb[i0 : i0 + S]
        wa = np.ascontiguousarray(
            ws.reshape(M9, 128, NU, US, IU).transpose(1, 0, 4, 3, 2)
        ).reshape(128, M9, IU * US, NU)
        xs = xb[:, :, i0 : i0 + S]
        xc = np.ascontiguousarray(
            xs.reshape(B, IU, M9, 128).transpose(3, 2, 1, 0)
        )
        xt = np.ascontiguousarray(xs)
        wb2 = np.ascontiguousarray(
            ws.reshape(M9, 128, NU, US, IU).transpose(1, 0, 4, 2, 3)
        ).reshape(128, M9, IU, SJ)
        in_maps.append({"wa": wa, "xc": xc, "xt": xt, "wb": wb2})
    return in_maps


def kernel(x, weight):
    from concourse.bass_utils import run_bass_kernel_spmd

    if "nc" not in _CACHE:
        _CACHE["nc"] = _build_program()
    in_maps = _shard_inputs(x, weight)
    res = run_bass_kernel_spmd(_CACHE["nc"], in_maps, list(range(N_CORES)))
    y = np.asarray(res.results[0]["y"], dtype=np.float32)
    return y.reshape(B, NU, US, 1)

